# revision 18
# baseline (speedup 1.0000x reference)
"""BloomAttention fused layer on 8 TRN2 NeuronCores (Bass/Tile SPMD), v3.

Strategy (tensor-parallel over heads):
  - 16 heads / 8 cores -> 2 heads per core.
  - QKV projection in bf16 (fp8 DoubleRow measured at the SAME per-row
    rate as bf16 on this part - power-throttled PE - so fp8 only cost
    accuracy; reverted).
  - q/k/v stay resident in SBUF (no DRAM round trip): the PSUM drains
    (ACT, +bias, alpha folded for q) write straight into persistent
    [128, head, 4096] tiles; V additionally transposed via PE into
    [key, d] blocks.
  - Attention per (batch, head): score tiles [128k, 512q] bf16 matmul,
    +ramp/mask on DVE, exp(+alibi bias) on ACT, denominator via
    ones-col PE matmul chain, 1/den as exp(-ln(den)) on ACT (same act
    table as Exp), context matmul chain, normalize on DVE; flush
    deferred one chunk so the recip hides under the next chunk's
    matmuls; rb PSUM->SBUF copy on DVE to keep ACT exp-only.
  - One AllGather per batch (both heads, [256,2048]bf16 = 1MB input)
    issued right after that batch's last context store; batch-0 gather
    hides under batch-1 attention, batch-1 gather under batch-0 dense.
  - Dense column-sharded: each core computes its 256 output columns
    with a single 16-step PSUM accumulation chain over the gathered
    context (cf rows are core-major so the contraction permutation is
    the identity); + residual + bias via one DVE add at drain.
  - All bulk loads are single rearranged DMAs (h: 1/chunk, w: 1, cx:
    1/chunk) - v2's 16-DMA loads serialized the Sync engine and
    starved the latency-critical context stores (38us PE stall).
"""

import math
import sys

sys.path.insert(0, "/opt/trn_rl_repo")

import ml_dtypes
import numpy as np

import concourse.bass as bass
import concourse.mybir as mybir
import concourse.tile as tile
from concourse.bass_utils import run_bass_kernel_spmd
from concourse.vector_clock import ScopedClock

# ---------------------------------------------------------------------------
# Workarounds for the walrus build in this container, which caps each
# instruction at ONE sync-wait command ("Too many sync wait commands" in
# CoreV3GenImpl setupSyncWait).
# ---------------------------------------------------------------------------
MAX_DRAIN_WAITS = 1


def _patched_drain_and_barrier(self, tick_clock, wait_clock):
    nc = self.nc
    drain_inst = nc.sync.drain()
    wait_clock.add_sem_waits(
        drain_inst.ins, ScopedClock({None: tick_clock.global_clock}))
    si = drain_inst.ins.sync_info
    waits = list(si.on_wait) if si is not None else []
    if len(waits) > MAX_DRAIN_WAITS:
        si.on_wait = waits[:MAX_DRAIN_WAITS]
        rest = waits[MAX_DRAIN_WAITS:]
        while rest:
            d2 = nc.sync.drain()
            si2 = d2.ins.sync_info
            if si2 is None:
                si2 = mybir.SyncInfo(on_wait=[], on_update=[])
                d2.ins.sync_info = si2
            si2.on_wait = rest[:MAX_DRAIN_WAITS]
            rest = rest[MAX_DRAIN_WAITS:]
    nc.all_engine_barrier()
    popped = nc._tile_sem_poison_stack.pop()
    assert popped is self._sem_poison
    nc.clear_and_free_semaphores(list(self.sems.allocated().values()))
    nc.all_engine_barrier()


tile.TileContext._drain_and_barrier = _patched_drain_and_barrier


def _split_multi_waits(nc, max_waits=1):
    """Move extra sync-waits onto standalone EventSemaphore (wait-only)
    instructions inserted just before the owner on the same engine --
    in-order issue preserves semantics exactly."""
    n = 0
    for fn in nc.m.functions:
        for blk in fn.blocks:
            new = []
            for inst in blk.instructions:
                si = inst.sync_info
                if si is not None and len(si.on_wait) > max_waits:
                    waits = list(si.on_wait)
                    for w in waits[:-max_waits]:
                        n += 1
                        new.append(mybir.InstEventSemaphore(
                            name=f"I-waitsplit-{n}",
                            opcode="EventSemaphore",
                            engine=inst.engine,
                            sync_info=mybir.SyncInfo(
                                on_wait=[w], on_update=[]),
                        ))
                    si.on_wait = waits[-max_waits:]
                new.append(inst)
            blk.instructions[:] = new
    return n


# ---------------------------------------------------------------------------

HIDDEN = 2048
N_HEAD = 16
HEAD_DIM = 128
B = 2
S = 2048
NTOK = B * S            # 4096 flattened tokens (batch-major)
N_CORES = 8
HPC = N_HEAD // N_CORES  # heads per core = 2
CPC = HPC * HEAD_DIM     # context rows per core = 256
ALPHA = 1.0 / math.sqrt(HEAD_DIM)

F32 = mybir.dt.float32
BF16 = mybir.dt.bfloat16
NP_BF16 = ml_dtypes.bfloat16

QC = 512                 # query-chunk (moving free dim)
KT = 128                 # key tile (partitions)
n_ht = HIDDEN // 128     # 16
n_kt = S // KT           # 16


def build_bass():
    nc = bass.Bass()

    # ---- per-core external inputs (bf16 for matmul operands) ----------
    hiddenT = nc.declare_dram_parameter("hiddenT", [HIDDEN, NTOK], BF16,
                                        isOutput=False)
    w_qkvT = nc.declare_dram_parameter("w_qkvT", [HIDDEN, 3 * CPC], BF16,
                                       isOutput=False)
    bvec = nc.declare_dram_parameter("bvec", [128, 3 * HPC], F32,
                                     isOutput=False)
    w_dT = nc.declare_dram_parameter("w_dT", [128, n_kt * CPC], BF16,
                                     isOutput=False)
    rpbT = nc.declare_dram_parameter("rpbT", [CPC, NTOK], BF16,
                                     isOutput=False)
    # alibi_qc[ki, (j,qc,kt)] = slope_j*(kt*128+ki) - slope_j*(qc*512)
    alibi = nc.declare_dram_parameter(
        "alibi", [128, HPC * (S // QC) * n_kt], F32, isOutput=False)
    # rmt[qi-part, j, m, q]: m=0 ramp -slope_j*qi; m=1+d ramp+mask delta d
    rmt = nc.declare_dram_parameter("rmt", [128, HPC, 5, QC], F32,
                                    isOutput=False)
    ident_in = nc.declare_dram_parameter("ident", [128, 128], BF16,
                                         isOutput=False)
    ones_in = nc.declare_dram_parameter("ones", [128], BF16,
                                        isOutput=False)
    out = nc.declare_dram_parameter("out", [CPC, NTOK], F32, isOutput=True)

    # ---- internal DRAM ------------------------------------------------
    # separate tensors per batch: no false WAR between a batch's context
    # stores and the other batch's in-flight AllGather.  Each batch's
    # gather is split into two token-half AllGathers (512KB input each)
    # triggered as soon as both heads have flushed that token half, so
    # the batch-1 gather tail hides under batch-0's dense.
    SH = S // 2
    ctx_d = [nc.dram_tensor(f"ctx_d{b}", [2, CPC, SH], BF16)
             for b in range(B)]
    cf = [[nc.dram_tensor(f"cf_{b}_{h}", [N_CORES * CPC, SH], BF16,
                          addr_space="Shared") for h in range(2)]
          for b in range(B)]

    with tile.TileContext(nc) as tc, nc.allow_low_precision(
            reason="bf16 matmul operands; fp32 accumulation throughout"):
        with tc.tile_pool(name="singles", bufs=1) as singles:
            # ---------- small constants first (cheap DMAs) ----------
            b_sb = singles.tile([128, 3 * HPC], F32)
            nc.sync.dma_start(out=b_sb, in_=bvec[:, :])
            ones_row = singles.tile([1, 128], BF16)
            nc.sync.dma_start(out=ones_row, in_=ones_in[None, :])
            ones_col = singles.tile([128, 1], BF16)
            nc.sync.dma_start(out=ones_col, in_=ones_in[:, None])
            ident = singles.tile([128, 128], BF16)
            nc.sync.dma_start(out=ident, in_=ident_in[:, :])
            alibi_sb = singles.tile([128, HPC * (S // QC) * n_kt], F32)
            nc.sync.dma_start(out=alibi_sb, in_=alibi[:, :])

            # persistent q/k/v in SBUF (written by QKV drains)
            qT_all = singles.tile([128, HPC, NTOK], BF16)
            kT_all = singles.tile([128, HPC, NTOK], BF16)
            v_all = singles.tile([128, HPC * 32, 128], BF16)
            # larger constants: issued after the first QKV weight/input
            # DMAs below so they don't delay the first matmul
            rmt_sb = singles.tile([128, HPC, 5, QC], F32)
            wd_sb = singles.tile([128, n_kt, CPC], BF16)

            # shared PSUM pools across all phases (8 banks total):
            # pps 4x[128,512]f32, pctx2 2, pmisc (aux+dacc) 2
            import contextlib
            _ps_stack = contextlib.ExitStack()
            pps = _ps_stack.enter_context(
                tc.tile_pool(name="pps", bufs=4, space="PSUM"))
            pctx2 = _ps_stack.enter_context(
                tc.tile_pool(name="pctx2", bufs=2, space="PSUM"))
            pmisc = _ps_stack.enter_context(
                tc.tile_pool(name="pmisc", bufs=1, space="PSUM"))

            # ---------- phase 1: QKV projection (+ V transpose) --------
            # col-tile ct (0..5) -> (head j=ct//3, part=ct%3); part 0=q
            # (scaled by ALPHA; bias pre-scaled on host), 1=k, 2=v.
            with (
                tc.tile_pool(name="wq", bufs=1) as wq,
                tc.tile_pool(name="hin", bufs=2) as hin,
                tc.tile_pool(name="proj", bufs=3) as proj,
            ):
                # w and h(0) split into per-ht-group pieces so the first
                # matmul chain starts ~10us in instead of waiting for the
                # full 5MB to land
                w_sb = wq.tile([128, n_ht, 3 * CPC], BF16)
                for ht in range(n_ht):
                    nc.sync.dma_start(
                        out=w_sb[:, ht, :],
                        in_=w_qkvT[ht * 128:(ht + 1) * 128, :])
                hidden_r = hiddenT.rearrange("(t p) n -> p t n", p=128)
                for tq in range(NTOK // QC):  # 8 token eighths
                    h_sb = hin.tile([128, n_ht, QC], BF16)
                    if tq == 0:
                        for g in range(4):
                            nc.sync.dma_start(
                                out=h_sb[:, g * 4:(g + 1) * 4, :],
                                in_=hidden_r[:, g * 4:(g + 1) * 4,
                                             0:QC])
                    else:
                        nc.sync.dma_start(
                            out=h_sb,
                            in_=hidden_r[:, :, tq * QC:(tq + 1) * QC])
                    if tq == 0:
                        # big constants can load now, behind w/h(0)
                        nc.sync.dma_start(out=rmt_sb, in_=rmt[:, :, :, :])
                        nc.sync.dma_start(
                            out=wd_sb,
                            in_=w_dT.rearrange("p (t n) -> p t n", t=n_kt))
                    for ct in range(3 * HPC):
                        ps = pps.tile([128, QC], F32, tag="ps")
                        for ht in range(n_ht):
                            nc.tensor.matmul(
                                ps,
                                w_sb[:, ht, ct * 128:(ct + 1) * 128],
                                h_sb[:, ht, :],
                                start=(ht == 0), stop=(ht == n_ht - 1))
                        j, part = divmod(ct, 3)
                        scale = ALPHA if part == 0 else 1.0
                        if part == 0:
                            o_dst = qT_all[:, j, tq * QC:(tq + 1) * QC]
                        elif part == 1:
                            o_dst = kT_all[:, j, tq * QC:(tq + 1) * QC]
                        else:
                            o_dst = proj.tile([128, QC], BF16, tag="v")
                        nc.scalar.activation(
                            o_dst, ps, mybir.ActivationFunctionType.Identity,
                            bias=b_sb[:, ct:ct + 1], scale=scale)
                        if part == 2:
                            # transpose vT [d, tok] -> V [tok, d] blocks
                            for i in range(QC // 128):
                                ps_t = pmisc.tile([128, 128], BF16,
                                                  tag="aux")
                                nc.tensor.transpose(
                                    ps_t, o_dst[:, i * 128:(i + 1) * 128],
                                    ident)
                                nc.vector.tensor_copy(
                                    v_all[:, j * 32 + tq * 4 + i, :], ps_t)

            # ---------- phase 2: attention + per-batch AllGather --------
            with (
                tc.tile_pool(name="sbuf_s", bufs=5) as sbuf_s,
                tc.tile_pool(name="ebuf", bufs=28) as ebuf,
                tc.tile_pool(name="cout", bufs=3) as cout,
                tc.tile_pool(name="dense", bufs=3) as dense,
                tc.tile_pool(name="dout", bufs=4) as dout,
            ):
                def gather_half(b, h):
                    nc.gpsimd.collective_compute(
                        "AllGather", mybir.AluOpType.bypass,
                        ins=[ctx_d[b][h, :, :]],
                        outs=[cf[b][h][:, :]],
                        replica_groups=[list(range(N_CORES))])

                def dense_chunk(bb, tc4):
                    # one token chunk of the dense projection: 2 chains of
                    # 16 accumulating matmuls over the gathered context
                    tcn = bb * (S // QC) + tc4
                    tl = (tc4 % 2) * QC      # offset within token half
                    cx_sb = dense_cx[bb][tc4]
                    for nt in range(CPC // 128):
                        rpb_sb = dout.tile([128, QC], BF16, tag="rpb")
                        nc.sync.dma_start(
                            out=rpb_sb,
                            in_=rpbT[nt * 128:(nt + 1) * 128,
                                     tcn * QC:(tcn + 1) * QC])
                        ps = pps.tile([128, QC], F32, tag="ps")
                        for kt in range(n_kt):
                            nc.tensor.matmul(
                                ps,
                                wd_sb[:, kt, nt * 128:(nt + 1) * 128],
                                cx_sb[:, kt, :],
                                start=(kt == 0), stop=(kt == n_kt - 1))
                        o_sb = dout.tile([128, QC], F32, tag="o")
                        nc.vector.tensor_add(o_sb, ps, rpb_sb)
                        nc.sync.dma_start(
                            out=out[nt * 128:(nt + 1) * 128,
                                    tcn * QC:(tcn + 1) * QC],
                            in_=o_sb)

                def load_cx(bb, tc4):
                    # prefetch one dense context chunk from the gathered
                    # cf half (issued early; waits on that half's gather)
                    h, off = divmod(tc4, 2)
                    cx_sb = dense.tile([128, n_kt, QC], BF16, tag="cx")
                    cf_r = cf[bb][h].rearrange("(t p) s -> p t s", p=128)
                    nc.sync.dma_start(
                        out=cx_sb,
                        in_=cf_r[:, :, off * QC:(off + 1) * QC])
                    dense_cx[bb][tc4] = cx_sb

                dense_cx = [[None] * (S // QC) for _ in range(B)]

                for b in range(B):
                    for j in range(HPC):
                        qT_sb = qT_all[:, j, b * S:(b + 1) * S]
                        kT_sb = kT_all[:, j, b * S:(b + 1) * S]

                        def flush(pend, b=b, j=j):
                            # normalize + store a finished chunk
                            pctx, precip, pqc = pend
                            rb = pmisc.tile([128, QC], F32, tag="aux")
                            nc.tensor.matmul(rb, ones_row, precip,
                                             start=True, stop=True)
                            rb_sb = cout.tile([128, QC], F32, tag="rbs")
                            nc.vector.tensor_copy(rb_sb, rb)
                            c_sb = cout.tile([128, QC], BF16, tag="c")
                            nc.vector.tensor_mul(c_sb, pctx, rb_sb)
                            h, off = divmod(pqc, 2)
                            nc.sync.dma_start(
                                out=ctx_d[b][h, j * 128:(j + 1) * 128,
                                             off * QC:(off + 1) * QC],
                                in_=c_sb)

                        # one PSUM bank, alternating rows per chunk: no WAR
                        # between chunks' denominators while recips drain
                        dacc4 = pmisc.tile([128, QC], F32, tag="dacc")
                        e_chunks = {}
                        pending = None

                        def emit_den_ctx(qc, b=b, j=j, dacc4=dacc4,
                                         e_chunks=e_chunks):
                            # denominator chain + recip + context chain
                            # for an already-exp'd chunk (cross-chunk
                            # pipelined: runs on PE while the NEXT
                            # chunk's exps stream on ACT)
                            kmax = (qc + 1) * (QC // KT)
                            e_tiles = e_chunks.pop(qc)
                            dacc = dacc4[(qc % 2) * 64:
                                         (qc % 2) * 64 + 1, :]
                            for kt in range(kmax):
                                nc.tensor.matmul(
                                    dacc, ones_col, e_tiles[kt],
                                    start=(kt == 0),
                                    stop=(kt == kmax - 1))
                            # 1/den on Scalar (exp(-ln(d)): same act table)
                            lden = cout.tile([1, QC], F32, tag="lden")
                            nc.scalar.activation(
                                lden, dacc,
                                mybir.ActivationFunctionType.Ln)
                            recip = cout.tile([1, QC], BF16, tag="recip")
                            nc.scalar.activation(
                                recip, lden,
                                mybir.ActivationFunctionType.Exp,
                                scale=-1.0)
                            ctx = pctx2.tile([128, QC], F32, tag="ctx")
                            for kt in range(kmax):
                                nc.tensor.matmul(
                                    ctx, v_all[:, j * 32 + b * 16 + kt, :],
                                    e_tiles[kt],
                                    start=(kt == 0),
                                    stop=(kt == kmax - 1))
                            return (ctx, recip, qc)

                        for qc in range(S // QC):
                            kmax = (qc + 1) * (QC // KT)
                            e_tiles = []
                            for kt in range(kmax):
                                ps = pps.tile([128, QC], F32, tag="ps")
                                nc.tensor.matmul(
                                    ps,
                                    kT_sb[:, kt * KT:(kt + 1) * KT],
                                    qT_sb[:, qc * QC:(qc + 1) * QC],
                                    start=True, stop=True)
                                delta = kt * KT - qc * QC
                                m = 0 if delta < 0 else 1 + delta // 128
                                s_sb = sbuf_s.tile([128, QC], F32, tag="s")
                                nc.vector.tensor_add(
                                    s_sb, ps, rmt_sb[:, j, m, :])
                                e_sb = ebuf.tile([128, QC], BF16, tag="e")
                                abase = (j * (S // QC) + qc) * n_kt + kt
                                nc.scalar.activation(
                                    e_sb, s_sb,
                                    mybir.ActivationFunctionType.Exp,
                                    bias=alibi_sb[:, abase:abase + 1])
                                e_tiles.append(e_sb)
                            e_chunks[qc] = e_tiles
                            if qc > 0:
                                done = emit_den_ctx(qc - 1)
                                if pending is not None:
                                    flush(pending)
                                    if j == 1 and qc == 3:
                                        # chunks 0-1 of both heads are
                                        # stored: first half can gather
                                        gather_half(b, 0)
                                pending = done
                        done = emit_den_ctx(S // QC - 1)
                        flush(pending)
                        flush(done)
                        if j == 1:
                            gather_half(b, 1)
                    if b == 0:
                        # prefetch batch-0 dense context during batch-1
                        # attention (DMAs wait on the gathers themselves)
                        for tc4 in range(S // QC):
                            load_cx(0, tc4)

                # ------ phase 3: dense (column shard) ----------------
                # cf rows are core-major: contraction tile kt covers
                # hidden rows kt*128..(kt+1)*128 (identity permutation)
                for tc4 in range(S // QC):
                    load_cx(1, tc4)
                for bb in range(B):
                    for tc4 in range(S // QC):
                        dense_chunk(bb, tc4)

            _ps_stack.close()

    _split_multi_waits(nc)
    return nc


def build_in_maps(hidden_states, residual, W_qkv, b_qkv, W_dense, b_dense):
    h2 = np.ascontiguousarray(
        hidden_states.reshape(NTOK, HIDDEN).T).astype(NP_BF16)
    rpb = (residual.reshape(NTOK, HIDDEN) + b_dense[None, :]).astype(
        np.float32)
    slopes = 2.0 ** (-8.0 * np.arange(1, N_HEAD + 1, dtype=np.float64)
                     / N_HEAD)
    pos = np.arange(S, dtype=np.float64)
    masks = np.zeros((4, KT, QC), np.float32)
    for d_i in range(4):
        d = d_i * 128
        ki = np.arange(KT)[:, None]
        qi = np.arange(QC)[None, :]
        masks[d_i] = np.where(ki + d > qi, np.float32(-10000.0), 0.0)

    w_dense_T = np.ascontiguousarray(W_dense.T)  # [hidden_in, hidden_out]

    in_maps = []
    for p in range(N_CORES):
        heads = [HPC * p + j for j in range(HPC)]
        w_qkv_p = W_qkv[p * 3 * CPC:(p + 1) * 3 * CPC, :]   # [768, 2048]
        w_qkvT = np.ascontiguousarray(w_qkv_p.T).astype(NP_BF16)
        bvec = np.zeros((3 * HPC, 128), np.float32)
        for ct in range(3 * HPC):
            j, part = divmod(ct, 3)
            seg = b_qkv[(heads[j] * 3 + part) * 128:
                        (heads[j] * 3 + part + 1) * 128]
            bvec[ct] = seg * (ALPHA if part == 0 else 1.0)
        bvec = np.ascontiguousarray(bvec.T)                # [128, 6]
        # dense: cf row kt*128+d == hidden row (head kt)*128+d: identity
        w_dT = np.ascontiguousarray(
            w_dense_T[:, p * CPC:(p + 1) * CPC]
            .reshape(n_kt, 128, CPC).transpose(1, 0, 2)
            .reshape(128, n_kt * CPC)).astype(NP_BF16)
        rpbT = np.ascontiguousarray(
            rpb[:, p * CPC:(p + 1) * CPC].T).astype(NP_BF16)
        al = np.zeros((HPC, S // QC, n_kt, KT), np.float64)
        rmtv = np.zeros((HPC, 5, 128, QC), np.float64)
        qi = np.arange(QC, dtype=np.float64)
        for j in range(HPC):
            sl = slopes[heads[j]]
            for qc in range(S // QC):
                al[j, qc] = (sl * pos).reshape(n_kt, KT) - sl * qc * QC
            ramp = np.broadcast_to(-sl * qi, (128, QC))
            rmtv[j, 0] = ramp
            for d_i in range(4):
                rmtv[j, 1 + d_i] = ramp + masks[d_i]
        al = np.ascontiguousarray(
            al.reshape(HPC * (S // QC) * n_kt, KT).T)      # [128, jct]
        rmtv = np.ascontiguousarray(rmtv.transpose(2, 0, 1, 3))  # [128,...]
        in_maps.append({
            "hiddenT": h2,
            "w_qkvT": w_qkvT,
            "bvec": bvec,
            "w_dT": w_dT,
            "rpbT": rpbT,
            "alibi": al.astype(np.float32),
            "rmt": rmtv.astype(np.float32),
            "ident": np.eye(128, dtype=NP_BF16),
            "ones": np.ones(128, dtype=NP_BF16),
        })
    return in_maps


_CACHED = {}


def kernel(hidden_states, residual, attention_mask, W_qkv, b_qkv,
           W_dense, b_dense, _profile=False, _tmpdir=None):
    del attention_mask  # all-ones in this problem
    in_maps = build_in_maps(np.asarray(hidden_states), np.asarray(residual),
                            np.asarray(W_qkv), np.asarray(b_qkv),
                            np.asarray(W_dense), np.asarray(b_dense))
    if "nc" not in _CACHED:
        _CACHED["nc"] = build_bass()
    nc = _CACHED["nc"]
    res = run_bass_kernel_spmd(
        nc, in_maps, core_ids=list(range(N_CORES)),
        trace=_profile, tmpdir=_tmpdir)
    shards = [res.results[p]["out"] for p in range(N_CORES)]
    full = np.concatenate(shards, axis=0)          # [2048 cols, 4096 tok]
    out = np.ascontiguousarray(full.T)             # [4096, 2048]
    if _profile:
        _CACHED["exec_time_ns"] = res.exec_time_ns
    return out.reshape(B, S, HIDDEN)


# revision 19
# speedup vs baseline: 1.1261x; 1.1261x over previous
"""BloomAttention fused layer on 8 TRN2 NeuronCores (Bass/Tile SPMD), v3.

Strategy (tensor-parallel over heads):
  - 16 heads / 8 cores -> 2 heads per core.
  - QKV projection in bf16 (fp8 DoubleRow measured at the SAME per-row
    rate as bf16 on this part - power-throttled PE - so fp8 only cost
    accuracy; reverted).
  - q/k/v stay resident in SBUF (no DRAM round trip): the PSUM drains
    (ACT, +bias, alpha folded for q) write straight into persistent
    [128, head, 4096] tiles; V additionally transposed via PE into
    [key, d] blocks.
  - Attention per (batch, head): score tiles [128k, 512q] bf16 matmul,
    +ramp/mask on DVE, exp(+alibi bias) on ACT, denominator via
    ones-col PE matmul chain, 1/den as exp(-ln(den)) on ACT (same act
    table as Exp), context matmul chain, normalize on DVE; flush
    deferred one chunk so the recip hides under the next chunk's
    matmuls; rb PSUM->SBUF copy on DVE to keep ACT exp-only.
  - One AllGather per batch (both heads, [256,2048]bf16 = 1MB input)
    issued right after that batch's last context store; batch-0 gather
    hides under batch-1 attention, batch-1 gather under batch-0 dense.
  - Dense column-sharded: each core computes its 256 output columns
    with a single 16-step PSUM accumulation chain over the gathered
    context (cf rows are core-major so the contraction permutation is
    the identity); + residual + bias via one DVE add at drain.
  - All bulk loads are single rearranged DMAs (h: 1/chunk, w: 1, cx:
    1/chunk) - v2's 16-DMA loads serialized the Sync engine and
    starved the latency-critical context stores (38us PE stall).
"""

import math
import sys

sys.path.insert(0, "/opt/trn_rl_repo")

import ml_dtypes
import numpy as np

import concourse.bass as bass
import concourse.mybir as mybir
import concourse.tile as tile
from concourse.bass_utils import run_bass_kernel_spmd
from concourse.vector_clock import ScopedClock

# ---------------------------------------------------------------------------
# Workarounds for the walrus build in this container, which caps each
# instruction at ONE sync-wait command ("Too many sync wait commands" in
# CoreV3GenImpl setupSyncWait).
# ---------------------------------------------------------------------------
MAX_DRAIN_WAITS = 1


def _patched_drain_and_barrier(self, tick_clock, wait_clock):
    nc = self.nc
    drain_inst = nc.sync.drain()
    wait_clock.add_sem_waits(
        drain_inst.ins, ScopedClock({None: tick_clock.global_clock}))
    si = drain_inst.ins.sync_info
    waits = list(si.on_wait) if si is not None else []
    if len(waits) > MAX_DRAIN_WAITS:
        si.on_wait = waits[:MAX_DRAIN_WAITS]
        rest = waits[MAX_DRAIN_WAITS:]
        while rest:
            d2 = nc.sync.drain()
            si2 = d2.ins.sync_info
            if si2 is None:
                si2 = mybir.SyncInfo(on_wait=[], on_update=[])
                d2.ins.sync_info = si2
            si2.on_wait = rest[:MAX_DRAIN_WAITS]
            rest = rest[MAX_DRAIN_WAITS:]
    nc.all_engine_barrier()
    popped = nc._tile_sem_poison_stack.pop()
    assert popped is self._sem_poison
    nc.clear_and_free_semaphores(list(self.sems.allocated().values()))
    nc.all_engine_barrier()


tile.TileContext._drain_and_barrier = _patched_drain_and_barrier


def _split_multi_waits(nc, max_waits=1):
    """Move extra sync-waits onto standalone EventSemaphore (wait-only)
    instructions inserted just before the owner on the same engine --
    in-order issue preserves semantics exactly."""
    n = 0
    for fn in nc.m.functions:
        for blk in fn.blocks:
            new = []
            for inst in blk.instructions:
                si = inst.sync_info
                if si is not None and len(si.on_wait) > max_waits:
                    waits = list(si.on_wait)
                    for w in waits[:-max_waits]:
                        n += 1
                        new.append(mybir.InstEventSemaphore(
                            name=f"I-waitsplit-{n}",
                            opcode="EventSemaphore",
                            engine=inst.engine,
                            sync_info=mybir.SyncInfo(
                                on_wait=[w], on_update=[]),
                        ))
                    si.on_wait = waits[-max_waits:]
                new.append(inst)
            blk.instructions[:] = new
    return n


# ---------------------------------------------------------------------------

HIDDEN = 2048
N_HEAD = 16
HEAD_DIM = 128
B = 2
S = 2048
NTOK = B * S            # 4096 flattened tokens (batch-major)
N_CORES = 8
HPC = N_HEAD // N_CORES  # heads per core = 2
CPC = HPC * HEAD_DIM     # context rows per core = 256
ALPHA = 1.0 / math.sqrt(HEAD_DIM)

F32 = mybir.dt.float32
BF16 = mybir.dt.bfloat16
FP8 = mybir.dt.float8e4
NP_BF16 = ml_dtypes.bfloat16

QC = 512                 # query-chunk (moving free dim)
KT = 128                 # key tile (partitions)
n_ht = HIDDEN // 128     # 16
n_kt = S // KT           # 16


def build_bass():
    nc = bass.Bass()

    # ---- per-core external inputs (bf16 for matmul operands) ----------
    hiddenT = nc.declare_dram_parameter("hiddenT", [HIDDEN, NTOK], BF16,
                                        isOutput=False)
    w_qkvT = nc.declare_dram_parameter("w_qkvT", [HIDDEN, 3 * CPC], BF16,
                                       isOutput=False)
    bvec = nc.declare_dram_parameter("bvec", [128, 3 * HPC], F32,
                                     isOutput=False)
    w_dT = nc.declare_dram_parameter("w_dT", [128, n_kt * CPC], BF16,
                                     isOutput=False)
    rpbT = nc.declare_dram_parameter("rpbT", [CPC, NTOK], BF16,
                                     isOutput=False)
    # alibi_qc[ki, (j,qc,kt)] = slope_j*(kt*128+ki) - slope_j*(qc*512)
    alibi = nc.declare_dram_parameter(
        "alibi", [128, HPC * (S // QC) * n_kt], F32, isOutput=False)
    # rmt[qi-part, j, m, q]: m=0 ramp -slope_j*qi; m=1+d ramp+mask delta d
    rmt = nc.declare_dram_parameter("rmt", [128, HPC, 5, QC], F32,
                                    isOutput=False)
    ident_in = nc.declare_dram_parameter("ident", [128, 128], BF16,
                                         isOutput=False)
    ones_in = nc.declare_dram_parameter("ones", [128], BF16,
                                        isOutput=False)
    out = nc.declare_dram_parameter("out", [CPC, NTOK], F32, isOutput=True)

    # ---- internal DRAM ------------------------------------------------
    # separate tensors per batch: no false WAR between a batch's context
    # stores and the other batch's in-flight AllGather.  Each batch's
    # gather is split into two token-half AllGathers (512KB input each)
    # triggered as soon as both heads have flushed that token half, so
    # the batch-1 gather tail hides under batch-0's dense.
    SH = S // 2
    ctx_d = [nc.dram_tensor(f"ctx_d{b}", [2, CPC, SH], FP8)
             for b in range(B)]
    cf = [[nc.dram_tensor(f"cf_{b}_{h}", [N_CORES * CPC, SH], FP8,
                          addr_space="Shared") for h in range(2)]
          for b in range(B)]

    with tile.TileContext(nc) as tc, nc.allow_low_precision(
            reason="bf16 matmul operands; fp32 accumulation throughout"):
        with tc.tile_pool(name="singles", bufs=1) as singles:
            # ---------- small constants first (cheap DMAs) ----------
            b_sb = singles.tile([128, 3 * HPC], F32)
            nc.sync.dma_start(out=b_sb, in_=bvec[:, :])
            ones_row = singles.tile([1, 128], BF16)
            nc.sync.dma_start(out=ones_row, in_=ones_in[None, :])
            ones_col = singles.tile([128, 1], BF16)
            nc.sync.dma_start(out=ones_col, in_=ones_in[:, None])
            ident = singles.tile([128, 128], BF16)
            nc.sync.dma_start(out=ident, in_=ident_in[:, :])
            alibi_sb = singles.tile([128, HPC * (S // QC) * n_kt], F32)
            nc.sync.dma_start(out=alibi_sb, in_=alibi[:, :])

            # persistent q/k/v in SBUF (written by QKV drains)
            qT_all = singles.tile([128, HPC, NTOK], BF16)
            kT_all = singles.tile([128, HPC, NTOK], BF16)
            v_all = singles.tile([128, HPC * 32, 128], BF16)
            # larger constants: issued after the first QKV weight/input
            # DMAs below so they don't delay the first matmul
            rmt_sb = singles.tile([128, HPC, 5, QC], F32)
            wd_sb = singles.tile([128, n_kt, CPC], BF16)

            # shared PSUM pools across all phases (8 banks total):
            # pps 4x[128,512]f32, pctx2 2, pmisc (aux+dacc) 2
            import contextlib
            _ps_stack = contextlib.ExitStack()
            pps = _ps_stack.enter_context(
                tc.tile_pool(name="pps", bufs=4, space="PSUM"))
            pctx2 = _ps_stack.enter_context(
                tc.tile_pool(name="pctx2", bufs=2, space="PSUM"))
            pmisc = _ps_stack.enter_context(
                tc.tile_pool(name="pmisc", bufs=1, space="PSUM"))

            # ---------- phase 1: QKV projection (+ V transpose) --------
            # col-tile ct (0..5) -> (head j=ct//3, part=ct%3); part 0=q
            # (scaled by ALPHA; bias pre-scaled on host), 1=k, 2=v.
            with (
                tc.tile_pool(name="wq", bufs=1) as wq,
                tc.tile_pool(name="hin", bufs=2) as hin,
                tc.tile_pool(name="proj", bufs=3) as proj,
            ):
                # w and h(0) split into per-ht-group pieces so the first
                # matmul chain starts ~10us in instead of waiting for the
                # full 5MB to land
                w_sb = wq.tile([128, n_ht, 3 * CPC], BF16)
                for ht in range(n_ht):
                    nc.sync.dma_start(
                        out=w_sb[:, ht, :],
                        in_=w_qkvT[ht * 128:(ht + 1) * 128, :])
                hidden_r = hiddenT.rearrange("(t p) n -> p t n", p=128)
                for tq in range(NTOK // QC):  # 8 token eighths
                    h_sb = hin.tile([128, n_ht, QC], BF16)
                    if tq == 0:
                        for g in range(4):
                            nc.sync.dma_start(
                                out=h_sb[:, g * 4:(g + 1) * 4, :],
                                in_=hidden_r[:, g * 4:(g + 1) * 4,
                                             0:QC])
                    else:
                        nc.sync.dma_start(
                            out=h_sb,
                            in_=hidden_r[:, :, tq * QC:(tq + 1) * QC])
                    if tq == 0:
                        # big constants can load now, behind w/h(0)
                        nc.sync.dma_start(out=rmt_sb, in_=rmt[:, :, :, :])
                        nc.sync.dma_start(
                            out=wd_sb,
                            in_=w_dT.rearrange("p (t n) -> p t n", t=n_kt))
                    for ct in range(3 * HPC):
                        ps = pps.tile([128, QC], F32, tag="ps")
                        for ht in range(n_ht):
                            nc.tensor.matmul(
                                ps,
                                w_sb[:, ht, ct * 128:(ct + 1) * 128],
                                h_sb[:, ht, :],
                                start=(ht == 0), stop=(ht == n_ht - 1))
                        j, part = divmod(ct, 3)
                        scale = ALPHA if part == 0 else 1.0
                        if part == 0:
                            o_dst = qT_all[:, j, tq * QC:(tq + 1) * QC]
                        elif part == 1:
                            o_dst = kT_all[:, j, tq * QC:(tq + 1) * QC]
                        else:
                            o_dst = proj.tile([128, QC], BF16, tag="v")
                        nc.scalar.activation(
                            o_dst, ps, mybir.ActivationFunctionType.Identity,
                            bias=b_sb[:, ct:ct + 1], scale=scale)
                        if part == 2:
                            # transpose vT [d, tok] -> V [tok, d] blocks
                            for i in range(QC // 128):
                                ps_t = pmisc.tile([128, 128], BF16,
                                                  tag="aux")
                                nc.tensor.transpose(
                                    ps_t, o_dst[:, i * 128:(i + 1) * 128],
                                    ident)
                                nc.vector.tensor_copy(
                                    v_all[:, j * 32 + tq * 4 + i, :], ps_t)

            # ---------- phase 2: attention + per-batch AllGather --------
            with (
                tc.tile_pool(name="sbuf_s", bufs=8) as sbuf_s,
                tc.tile_pool(name="ebuf", bufs=30) as ebuf,
                tc.tile_pool(name="cout", bufs=6) as cout,
                tc.tile_pool(name="dense", bufs=3) as dense,
                tc.tile_pool(name="dout", bufs=4) as dout,
            ):
                def gather_half(b, h):
                    nc.gpsimd.collective_compute(
                        "AllGather", mybir.AluOpType.bypass,
                        ins=[ctx_d[b][h, :, :]],
                        outs=[cf[b][h][:, :]],
                        replica_groups=[list(range(N_CORES))])

                def dense_chunk(bb, tc4):
                    # one token chunk of the dense projection: 2 chains of
                    # 16 accumulating matmuls over the gathered context
                    tcn = bb * (S // QC) + tc4
                    tl = (tc4 % 2) * QC      # offset within token half
                    cx_sb = dense_cx[bb][tc4]
                    for nt in range(CPC // 128):
                        rpb_sb = dout.tile([128, QC], BF16, tag="rpb")
                        nc.sync.dma_start(
                            out=rpb_sb,
                            in_=rpbT[nt * 128:(nt + 1) * 128,
                                     tcn * QC:(tcn + 1) * QC])
                        ps = pps.tile([128, QC], F32, tag="ps")
                        for kt in range(n_kt):
                            nc.tensor.matmul(
                                ps,
                                wd_sb[:, kt, nt * 128:(nt + 1) * 128],
                                cx_sb[:, kt, :],
                                start=(kt == 0), stop=(kt == n_kt - 1))
                        o_sb = dout.tile([128, QC], F32, tag="o")
                        nc.vector.tensor_add(o_sb, ps, rpb_sb)
                        nc.sync.dma_start(
                            out=out[nt * 128:(nt + 1) * 128,
                                    tcn * QC:(tcn + 1) * QC],
                            in_=o_sb)

                def load_cx(bb, tc4):
                    # prefetch one dense context chunk from the gathered
                    # cf half (issued early; waits on that half's gather)
                    h, off = divmod(tc4, 2)
                    cx_sb = dense.tile([128, n_kt, QC], FP8, tag="cx")
                    cf_r = cf[bb][h].rearrange("(t p) s -> p t s", p=128)
                    nc.sync.dma_start(
                        out=cx_sb,
                        in_=cf_r[:, :, off * QC:(off + 1) * QC])
                    dense_cx[bb][tc4] = cx_sb

                dense_cx = [[None] * (S // QC) for _ in range(B)]

                for b in range(B):
                    for j in range(HPC):
                        qT_sb = qT_all[:, j, b * S:(b + 1) * S]
                        kT_sb = kT_all[:, j, b * S:(b + 1) * S]

                        def flush(pend, b=b, j=j):
                            # normalize + store a finished chunk
                            pctx, precip, pqc = pend
                            rb = pmisc.tile([128, QC], F32, tag="aux")
                            nc.tensor.matmul(rb, ones_row, precip,
                                             start=True, stop=True)
                            rb_sb = cout.tile([128, QC], F32, tag="rbs")
                            nc.vector.tensor_copy(rb_sb, rb)
                            c_sb = cout.tile([128, QC], FP8, tag="c")
                            nc.vector.tensor_mul(c_sb, pctx, rb_sb)
                            h, off = divmod(pqc, 2)
                            nc.sync.dma_start(
                                out=ctx_d[b][h, j * 128:(j + 1) * 128,
                                             off * QC:(off + 1) * QC],
                                in_=c_sb)

                        # one PSUM bank, alternating rows per chunk: no WAR
                        # between chunks' denominators while recips drain
                        dacc4 = pmisc.tile([128, QC], F32, tag="dacc")
                        e_chunks = {}
                        pending = None

                        def emit_den_ctx(qc, b=b, j=j, dacc4=dacc4,
                                         e_chunks=e_chunks):
                            # denominator chain + recip + context chain
                            # for an already-exp'd chunk (cross-chunk
                            # pipelined: runs on PE while the NEXT
                            # chunk's exps stream on ACT)
                            kmax = (qc + 1) * (QC // KT)
                            e_tiles = e_chunks.pop(qc)
                            dacc = dacc4[(qc % 2) * 64:
                                         (qc % 2) * 64 + 1, :]
                            for kt in range(kmax):
                                nc.tensor.matmul(
                                    dacc, ones_col, e_tiles[kt],
                                    start=(kt == 0),
                                    stop=(kt == kmax - 1))
                            # 1/den on Scalar (exp(-ln(d)): same act table)
                            lden = cout.tile([1, QC], F32, tag="lden")
                            nc.scalar.activation(
                                lden, dacc,
                                mybir.ActivationFunctionType.Ln)
                            recip = cout.tile([1, QC], BF16, tag="recip")
                            nc.scalar.activation(
                                recip, lden,
                                mybir.ActivationFunctionType.Exp,
                                scale=-1.0)
                            ctx = pctx2.tile([128, QC], F32, tag="ctx")
                            for kt in range(kmax):
                                nc.tensor.matmul(
                                    ctx, v_all[:, j * 32 + b * 16 + kt, :],
                                    e_tiles[kt],
                                    start=(kt == 0),
                                    stop=(kt == kmax - 1))
                            return (ctx, recip, qc)

                        for qc in range(S // QC):
                            kmax = (qc + 1) * (QC // KT)
                            e_tiles = []
                            for kt in range(kmax):
                                ps = pps.tile([128, QC], F32, tag="ps")
                                nc.tensor.matmul(
                                    ps,
                                    kT_sb[:, kt * KT:(kt + 1) * KT],
                                    qT_sb[:, qc * QC:(qc + 1) * QC],
                                    start=True, stop=True)
                                delta = kt * KT - qc * QC
                                m = 0 if delta < 0 else 1 + delta // 128
                                s_sb = sbuf_s.tile([128, QC], F32, tag="s")
                                nc.vector.tensor_add(
                                    s_sb, ps, rmt_sb[:, j, m, :])
                                e_sb = ebuf.tile([128, QC], BF16, tag="e")
                                abase = (j * (S // QC) + qc) * n_kt + kt
                                nc.scalar.activation(
                                    e_sb, s_sb,
                                    mybir.ActivationFunctionType.Exp,
                                    bias=alibi_sb[:, abase:abase + 1])
                                e_tiles.append(e_sb)
                            e_chunks[qc] = e_tiles
                            if qc > 0:
                                done = emit_den_ctx(qc - 1)
                                if pending is not None:
                                    flush(pending)
                                    if j == 1 and qc == 3:
                                        # chunks 0-1 of both heads are
                                        # stored: first half can gather
                                        gather_half(b, 0)
                                pending = done
                        done = emit_den_ctx(S // QC - 1)
                        flush(pending)
                        flush(done)
                        if j == 1:
                            gather_half(b, 1)
                    if b == 0:
                        # prefetch batch-0 dense context during batch-1
                        # attention (DMAs wait on the gathers themselves)
                        for tc4 in range(S // QC):
                            load_cx(0, tc4)

                # ------ phase 3: dense (column shard) ----------------
                # cf rows are core-major: contraction tile kt covers
                # hidden rows kt*128..(kt+1)*128 (identity permutation)
                for tc4 in range(S // QC):
                    dense_chunk(0, tc4)
                load_cx(1, 0)
                load_cx(1, 1)
                dense_chunk(1, 0)
                load_cx(1, 2)
                dense_chunk(1, 1)
                load_cx(1, 3)
                dense_chunk(1, 2)
                dense_chunk(1, 3)

            _ps_stack.close()

    _split_multi_waits(nc)
    return nc


def build_in_maps(hidden_states, residual, W_qkv, b_qkv, W_dense, b_dense):
    h2 = np.ascontiguousarray(
        hidden_states.reshape(NTOK, HIDDEN).T).astype(NP_BF16)
    rpb = (residual.reshape(NTOK, HIDDEN) + b_dense[None, :]).astype(
        np.float32)
    slopes = 2.0 ** (-8.0 * np.arange(1, N_HEAD + 1, dtype=np.float64)
                     / N_HEAD)
    pos = np.arange(S, dtype=np.float64)
    masks = np.zeros((4, KT, QC), np.float32)
    for d_i in range(4):
        d = d_i * 128
        ki = np.arange(KT)[:, None]
        qi = np.arange(QC)[None, :]
        masks[d_i] = np.where(ki + d > qi, np.float32(-10000.0), 0.0)

    w_dense_T = np.ascontiguousarray(W_dense.T)  # [hidden_in, hidden_out]

    in_maps = []
    for p in range(N_CORES):
        heads = [HPC * p + j for j in range(HPC)]
        w_qkv_p = W_qkv[p * 3 * CPC:(p + 1) * 3 * CPC, :]   # [768, 2048]
        w_qkvT = np.ascontiguousarray(w_qkv_p.T).astype(NP_BF16)
        bvec = np.zeros((3 * HPC, 128), np.float32)
        for ct in range(3 * HPC):
            j, part = divmod(ct, 3)
            seg = b_qkv[(heads[j] * 3 + part) * 128:
                        (heads[j] * 3 + part + 1) * 128]
            bvec[ct] = seg * (ALPHA if part == 0 else 1.0)
        bvec = np.ascontiguousarray(bvec.T)                # [128, 6]
        # dense: cf row kt*128+d == hidden row (head kt)*128+d: identity
        w_dT = np.ascontiguousarray(
            w_dense_T[:, p * CPC:(p + 1) * CPC]
            .reshape(n_kt, 128, CPC).transpose(1, 0, 2)
            .reshape(128, n_kt * CPC)).astype(NP_BF16)
        rpbT = np.ascontiguousarray(
            rpb[:, p * CPC:(p + 1) * CPC].T).astype(NP_BF16)
        al = np.zeros((HPC, S // QC, n_kt, KT), np.float64)
        rmtv = np.zeros((HPC, 5, 128, QC), np.float64)
        qi = np.arange(QC, dtype=np.float64)
        for j in range(HPC):
            sl = slopes[heads[j]]
            for qc in range(S // QC):
                al[j, qc] = (sl * pos).reshape(n_kt, KT) - sl * qc * QC
            ramp = np.broadcast_to(-sl * qi, (128, QC))
            rmtv[j, 0] = ramp
            for d_i in range(4):
                rmtv[j, 1 + d_i] = ramp + masks[d_i]
        al = np.ascontiguousarray(
            al.reshape(HPC * (S // QC) * n_kt, KT).T)      # [128, jct]
        rmtv = np.ascontiguousarray(rmtv.transpose(2, 0, 1, 3))  # [128,...]
        in_maps.append({
            "hiddenT": h2,
            "w_qkvT": w_qkvT,
            "bvec": bvec,
            "w_dT": w_dT,
            "rpbT": rpbT,
            "alibi": al.astype(np.float32),
            "rmt": rmtv.astype(np.float32),
            "ident": np.eye(128, dtype=NP_BF16),
            "ones": np.ones(128, dtype=NP_BF16),
        })
    return in_maps


_CACHED = {}


def kernel(hidden_states, residual, attention_mask, W_qkv, b_qkv,
           W_dense, b_dense, _profile=False, _tmpdir=None):
    del attention_mask  # all-ones in this problem
    in_maps = build_in_maps(np.asarray(hidden_states), np.asarray(residual),
                            np.asarray(W_qkv), np.asarray(b_qkv),
                            np.asarray(W_dense), np.asarray(b_dense))
    if "nc" not in _CACHED:
        _CACHED["nc"] = build_bass()
    nc = _CACHED["nc"]
    res = run_bass_kernel_spmd(
        nc, in_maps, core_ids=list(range(N_CORES)),
        trace=_profile, tmpdir=_tmpdir)
    shards = [res.results[p]["out"] for p in range(N_CORES)]
    full = np.concatenate(shards, axis=0)          # [2048 cols, 4096 tok]
    out = np.ascontiguousarray(full.T)             # [4096, 2048]
    if _profile:
        _CACHED["exec_time_ns"] = res.exec_time_ns
    return out.reshape(B, S, HIDDEN)
